# revision 36
# baseline (speedup 1.0000x reference)
"""Trainium2 Bass kernel for nn_MultiHeadAttn, v5: one pipelined hw loop.

The relay executing NEFFs in this environment charges wall time per
EMITTED instruction (decode/dispatch); executed loop iterations are
nearly free.  v4 wrapped every phase in its own tc.For_i (v3 left the
attention loop unrolled); v5 fuses all three phases into ONE mega-loop
via a 3-stage cross-rep software pipeline, paying the For_i framework
overhead (skip/reset/exit sync blocks, ~85 instructions) once instead
of three times:

  mega-loop (4 iters, rep r): qkv(r) + attention(r-1) + out-proj(r-2)
      qkv:  one DMA stages both batches' QK rhs slabs + the V lhsT
            slab; 48 matmuls; V results DMA'd straight into the
            AllToAll#1 input at a loop-affine offset.
      attn: full-partition dynamic-src copies stage qk/v, then a
            STATIC partition-shifted extraction of k (partition-
            shifted copies with a dynamic offset on EITHER side
            corrupt -- verified on hardware; symbolic matmul rhs
            costs +1 register-math ISA instruction per matmul);
            causal flash attention (40+40 matmuls, exp-bias trick,
            flash ones-column row sums); normalized AV DMA'd into the
            AllToAll#2 input at a loop-affine offset.
      out:  out-proj (16 matmuls) + residual + per-tile LN.
  between mega-loops: AllToAll#1(r) + vext gather, AllToAll#2(r-1) +
      avg gather.  qk_pack/vext are double-buffered (rep parity);
      trailing attn+out loops redo the final rep so out_d is always
      correct, including the real repeat=1 call.

Host-side layout prep makes every gather/scatter collapse to <=3-dim
DMA access patterns (the hard DMA limit) and every loop-varying offset
affine in the loop variable:
  hqv:  [4 groups g, 128, 2*KCH*512 + KCH*128]  (qk rhs + v lhsT slabs)
  a2a1 chunk layout [i2, b, tok, colh]: slab offset affine in vt'=2*i2+b,
      and jt = 2*src + i2 becomes a single 32768-stride gather dim.
  a2a2 chunk layout [u, d, tok]: offset affine in u; (h,d) partition
      dims merge into one 256-stride dim for the avg gather.
"""

import os
import sys

import numpy as np

try:
    import concourse.bass as bass  # noqa: F401
except ImportError:  # pragma: no cover
    sys.path.insert(0, "/opt/trn_rl_repo")

import ml_dtypes

import concourse.bass as bass
from concourse.bass import ds
import concourse.mybir as mybir
import concourse.tile as tile
from concourse import bacc
from concourse.bass_utils import run_bass_kernel_spmd
from concourse.masks import make_upper_triangular

# Problem constants
T_FULL = 2048
B = 2
D_MODEL = 1024
N_HEAD = 16
D_HEAD = 64
LN_EPS = 1e-5
N_CORES = 8
SCALE = 1.0 / (D_HEAD**0.5)
EXP_BIAS = -3.0  # scores are in [-3.3, 3.3] for this problem; keeps exp <= ~1.4

P = 128
KCH = D_MODEL // P  # 8 contraction chunks
IB = 512  # query block width
JG = 4  # j-tiles per score/exp group (group = [128, 4, 512] = 4 PSUM banks)

F32 = mybir.dt.float32
BF16 = mybir.dt.bfloat16

LAST_RESULT = None


def _ap(t, offset_elems, dims):
    """Hand-built access pattern: dims = [(stride, size), ...] in elements.

    offset_elems may be a loop-variable expression (symbolic AP)."""
    return bass.AP(tensor=t.tensor, offset=offset_elems + t.offset, ap=[list(d) for d in dims])


def build_program(t=T_FULL, n_cores=N_CORES, repeat=1, no_collective=False, apply_gb=True,
                  hw_loops=("qkv", "attn", "out"), pipeline_out=True,
                  ):
    nh_loc = N_HEAD // n_cores  # 2 heads per core
    assert nh_loc == 2 and B == 2
    nt = t // P  # 16 j-tiles per batch
    n_ib = t // IB  # 4 query blocks per batch
    n_g = t // IB  # 4 token slabs per batch for qk proj
    cs = t // n_cores  # 256 tokens per batch per core
    tiles_pb = cs // P  # 2 output tiles per batch
    n_it = B * tiles_pb  # 4 output tiles per core
    n_u = B * nh_loc  # 4 attention units per core
    SLAB_Q = P * KCH * IB  # elements per hq slab
    SLAB_V = P * KCH * P  # elements per hv slab
    CH1 = tiles_pb * B * P * P  # a2a1 per-chunk elements (65536)
    CH2 = n_u * D_HEAD * cs  # a2a2 per-chunk elements (65536)

    nc = bacc.Bacc("TRN2", target_bir_lowering=False, debug=False, num_devices=n_cores)

    def loop(tc, tag, n, name, body):
        if tag in hw_loops:
            with tc.For_i(0, n, name=name) as v:
                body(v)
        else:
            for v in range(n):
                body(v)

    # Kernel I/O
    # hqv: per token-block g, both batches' qk-proj rhs slabs followed by
    # the v-proj lhsT slab, so one in-loop DMA stages all matmul operands.
    QV_W = B * KCH * IB + KCH * P  # 9216 columns per slab group
    hqv_d = nc.dram_tensor("hqv", [n_g, P, QV_W], BF16, kind="ExternalInput").ap()
    wqk_d = nc.dram_tensor("wqk", [KCH, P, 2 * nh_loc * D_HEAD], BF16, kind="ExternalInput").ap()
    wv_d = nc.dram_tensor("wv", [KCH, P, D_MODEL], BF16, kind="ExternalInput").ap()
    wo_d = nc.dram_tensor("wo", [KCH, P, D_MODEL], BF16, kind="ExternalInput").ap()
    hres_d = nc.dram_tensor("hres", [n_it, P, D_MODEL], F32, kind="ExternalInput").ap()
    g_d = nc.dram_tensor("lng", [D_MODEL], F32, kind="ExternalInput").ap()
    b_d = nc.dram_tensor("lnb", [D_MODEL], F32, kind="ExternalInput").ap()
    out_d = nc.dram_tensor("out", [n_it, P, D_MODEL], F32, kind="ExternalOutput").ap()

    with tile.TileContext(nc) as tc:
        with (
            tc.tile_pool(name="consts", bufs=1) as consts,
            tc.tile_pool(name="sb", bufs=1) as sb,
            tc.tile_pool(name="pproj", bufs=1, space="PSUM") as pproj,
            tc.tile_pool(name="psc", bufs=1, space="PSUM") as psc,
            tc.tile_pool(name="pav", bufs=2, space="PSUM") as pav,
            tc.tile_pool(name="dram", bufs=1, space="DRAM") as dram,
        ):
            # ---- constants ----
            wqk_sb = consts.tile([P, KCH, 2 * nh_loc * D_HEAD], BF16)
            nc.sync.dma_start(
                out=wqk_sb,
                in_=_ap(wqk_d, 0, [(256, P), (P * 256, KCH), (1, 256)]),
            )
            wv_sb = consts.tile([P, KCH, D_MODEL], BF16)
            nc.sync.dma_start(
                out=wv_sb,
                in_=_ap(wv_d, 0, [(D_MODEL, P), (P * D_MODEL, KCH), (1, D_MODEL)]),
            )
            wo_sb = consts.tile([P, KCH, D_MODEL], BF16)
            nc.sync.dma_start(
                out=wo_sb,
                in_=_ap(wo_d, 0, [(D_MODEL, P), (P * D_MODEL, KCH), (1, D_MODEL)]),
            )
            if apply_gb:
                g_sb = consts.tile([P, D_MODEL], F32)
                b_sb = consts.tile([P, D_MODEL], F32)
                nc.sync.dma_start(out=g_sb, in_=_ap(g_d, 0, [(0, P), (1, D_MODEL)]))
                nc.sync.dma_start(out=b_sb, in_=_ap(b_d, 0, [(0, P), (1, D_MODEL)]))

            eps_sb = consts.tile([P, 1], F32)
            nc.vector.memset(eps_sb, LN_EPS)
            expb_sb = consts.tile([P, 1], F32)
            nc.vector.memset(expb_sb, EXP_BIAS)

            # grouped causal mask m4[:, g, :]: cols [0, g*128) = 0 (future keys),
            # [g*128, (g+1)*128) upper-triangular-incl-diag, rest = 1
            m4 = consts.tile([P, JG, IB], BF16)
            nc.gpsimd.memset(m4, 1.0)
            for g in range(1, JG):
                nc.gpsimd.memset(m4[:, g, 0 : g * P], 0.0)
            for g in range(JG):
                make_upper_triangular(nc, m4[:, g, g * P : (g + 1) * P], val=1.0, diag=True)

            # ---- tiles (declared once; buffers are reused across reps) ----
            a2a1_in = dram.tile([n_cores, tiles_pb, B, P, P], BF16, name="a2a1_in")
            a2a1_out = dram.tile([n_cores, tiles_pb, B, P, P], BF16, name="a2a1_out")
            a2a2_in = dram.tile([n_cores, n_u, D_HEAD, cs], BF16, name="a2a2_in")
            a2a2_out = dram.tile([n_cores, n_u, D_HEAD, cs], BF16, name="a2a2_out")

            # qk_pack: q on partitions 0:64, k on 64:128; flat (b, h) unit
            # dim.  Two explicit buffers: the mega-loop of rep r writes
            # qk_pk[r%2] while rep r-1's attention reads qk_pk[(r+1)%2].
            qk_pk = [sb.tile([P, n_u, t], BF16, tag=f"qkp{j}", name=f"qk_pack{j}")
                     for j in range(2)]
            vext_pk = [sb.tile([P, nt, n_u, D_HEAD + 1], BF16, tag=f"vext{j}",
                               name=f"vext_all{j}") for j in range(2)]
            hqv_t = sb.tile([P, QV_W], BF16, tag="hqv", name="hqv_t")
            vsm = sb.tile([P, D_MODEL], BF16, tag="vsm", name="vsm")
            V_OFF = B * KCH * IB  # v slab column offset within hqv_t

            # attention stages in F32: an f32-ifmap Matmult is self-loading,
            # so every f32 matmul saves the separate Ldweights instruction
            qkstage = sb.tile([P, t], F32, tag="qks", name="qkstage")
            kst = sb.tile([D_HEAD, t], F32, tag="kst", name="kst")
            vstage = sb.tile([P, nt, D_HEAD + 1], F32, tag="vst", name="vstage")
            avu = sb.tile([D_HEAD + 1, t], F32, tag="avu", name="avu")
            srow = sb.tile([1, t], F32, tag="srow", name="srow")
            rb = sb.tile([D_HEAD, t], F32, tag="rb", name="rb")
            avt = sb.tile([D_HEAD, t], BF16, tag="avt", name="avt")

            avg_sb = sb.tile([P, n_cores, B * cs], BF16, tag="avg", name="avg_sb")
            ostage = sb.tile([P, n_cores, P], BF16, tag="ost", name="ostage")
            hres_t = sb.tile([P, D_MODEL], F32, tag="hrt", name="hres_t")
            x_t = sb.tile([P, D_MODEL], F32, tag="x", name="x_t")
            # one tile for all LN scalars (fewer tags -> fewer per-loop
            # semaphore resets): [0:12] bn_stats x2, [12:14] (mean, var),
            # [14:15] std, [15:16] rstd
            lns = sb.tile([P, 16], F32, tag="lns", name="lns")

            # ones columns for the flash row-sum trick (gathers leave col 64
            # untouched), and defined values everywhere the pipelined rep-0
            # reads not-yet-produced data (results are recomputed by the
            # trailing loops, so out_d always ends correct)
            for j in range(2):
                nc.vector.memset(vext_pk[j], 0.0)
                nc.vector.memset(vext_pk[j][:, :, :, D_HEAD : D_HEAD + 1], 1.0)
                if pipeline_out:
                    nc.vector.memset(qk_pk[j], 0.0)
            if pipeline_out:
                nc.vector.memset(avg_sb, 0.0)

            def out_body(it):
                # tile it = (b, i2) covers avg cols [it*128, (it+1)*128)
                nc.vector.tensor_copy(ostage, avg_sb[:, :, ds(it * P, P)])
                nc.sync.dma_start(out=hres_t, in_=hres_d[ds(it, 1)])
                pos = pproj.tile([P, D_MODEL], F32, tag="proj", name="pos")
                for nh in range(2):
                    for k in range(n_cores):
                        nc.tensor.matmul(
                            pos[:, nh * IB : (nh + 1) * IB],
                            lhsT=ostage[:, k, :],
                            rhs=wo_sb[:, k, nh * IB : (nh + 1) * IB],
                            start=(k == 0),
                            stop=(k == n_cores - 1),
                        )
                nc.vector.tensor_add(x_t, pos, hres_t)
                for s in range(2):
                    nc.vector.bn_stats(lns[:, s * 6 : (s + 1) * 6], x_t[:, s * IB : (s + 1) * IB])
                nc.vector.bn_aggr(lns[:, 12:14], lns[:, 0:12])
                nc.scalar.activation(
                    lns[:, 14:15], lns[:, 13:14], mybir.ActivationFunctionType.Sqrt, bias=eps_sb
                )
                nc.vector.reciprocal(lns[:, 15:16], lns[:, 14:15])
                nc.vector.tensor_scalar(
                    out=x_t,
                    in0=x_t,
                    scalar1=lns[:, 12:13],
                    scalar2=lns[:, 15:16],
                    op0=mybir.AluOpType.subtract,
                    op1=mybir.AluOpType.mult,
                )
                if apply_gb:
                    nc.vector.tensor_mul(x_t, x_t, g_sb)
                    nc.vector.tensor_add(x_t, x_t, b_sb)
                nc.sync.dma_start(out=out_d[ds(it, 1)], in_=x_t)

            def qkv_body(g, qk_pack):
                # stage both batches' qk rhs slabs + the v lhsT slab, one DMA
                nc.sync.dma_start(
                    out=hqv_t,
                    in_=_ap(hqv_d, g * (P * QV_W), [(QV_W, P), (1, QV_W)]),
                )
                for b in range(B):
                    ps = pproj.tile([P, 2, IB], F32, tag="proj", name=f"ps_qk{b}")
                    # wqk columns are per-head [q|k] blocks, so ps[:, h, :]
                    # lands with q on partitions 0:64 and k on 64:128.
                    for mt in range(2):
                        for k in range(KCH):
                            nc.tensor.matmul(
                                ps[:, mt, :],
                                lhsT=wqk_sb[:, k, mt * P : (mt + 1) * P],
                                rhs=hqv_t[:, b * KCH * IB + k * IB : b * KCH * IB + (k + 1) * IB],
                                start=(k == 0),
                                stop=(k == KCH - 1),
                            )
                    nc.vector.tensor_copy(
                        qk_pack[:, b * nh_loc : (b + 1) * nh_loc, ds(g * IB, IB)],
                        ps,
                    )
                # V projection for slab vt' = g = 2*i2 + b
                psv = pproj.tile([P, 2, IB], F32, tag="proj", name="ps_v")
                for nh in range(2):
                    for k in range(KCH):
                        nc.tensor.matmul(
                            psv[:, nh, :],
                            lhsT=hqv_t[:, V_OFF + k * P : V_OFF + (k + 1) * P],
                            rhs=wv_sb[:, k, nh * IB : (nh + 1) * IB],
                            start=(k == 0),
                            stop=(k == KCH - 1),
                        )
                nc.vector.tensor_copy(vsm, psv)
                # straight into the AllToAll#1 input: chunk layout
                # [i2, b, tok, colh] -> slab offset = 16384 * vt'
                nc.sync.dma_start(
                    out=_ap(a2a1_in, g * (P * P),
                            [(P, P), (CH1, n_cores), (1, P)]),
                    in_=vsm,
                )

            def attn_u_body(u, qk_pack, vext_all):
                # full-partition dynamic-src stage, then STATIC partition-
                # shifted extraction of k.  Partition-shifted copies with a
                # dynamic offset on EITHER side corrupt (verified), and a
                # symbolic matmul rhs costs one register-math ISA
                # instruction per matmul, so q is read from the stage.
                nc.vector.tensor_copy(qkstage, qk_pack[:, ds(u, 1), :])
                nc.vector.tensor_copy(kst, qkstage[D_HEAD:P, :])
                nc.vector.tensor_copy(vstage, vext_all[:, :, ds(u, 1), :])
                for ib in range(n_ib):
                    avps = pav.tile([D_HEAD + 1, IB], F32, tag="av", name="avps")
                    for grp in range(ib + 1):
                        scp = psc.tile([P, JG, IB], F32, tag="sc", name="scp")
                        for jj in range(JG):
                            jt = JG * grp + jj
                            nc.tensor.matmul(
                                scp[:, jj, :],
                                lhsT=kst[:, jt * P : (jt + 1) * P],
                                rhs=qkstage[0:D_HEAD, ib * IB : (ib + 1) * IB],
                                start=True,
                                stop=True,
                            )
                        expt = sb.tile([P, JG, IB], F32, tag="exp", name="expt")
                        nc.scalar.activation(
                            expt, scp, mybir.ActivationFunctionType.Exp, bias=expb_sb
                        )
                        if grp == ib:
                            nc.vector.tensor_mul(expt, expt, m4)
                        for jj in range(JG):
                            jt = JG * grp + jj
                            nc.tensor.matmul(
                                avps,
                                lhsT=vstage[:, jt, :],
                                rhs=expt[:, jj, :],
                                start=(jt == 0),
                                stop=(jt == JG * ib + JG - 1),
                            )
                    nc.vector.tensor_copy(avu[:, ib * IB : (ib + 1) * IB], avps)
                # partition-shifted reads feed custom-DVE ops wrongly; a
                # plain tensor_copy handles the shift
                nc.vector.tensor_copy(srow, avu[D_HEAD : D_HEAD + 1, :])
                nc.vector.reciprocal_approx_fast(out=rb[0:1, :], in_=srow)
                nc.gpsimd.partition_broadcast(rb, rb[0:1, :])
                nc.vector.tensor_mul(avt, avu[0:D_HEAD, :], rb)
                # straight into the AllToAll#2 input: chunk layout
                # [u, d, tok] -> offset = u * D_HEAD * cs; token blocks of
                # 256 pair with destination cores
                nc.sync.dma_start(
                    out=_ap(a2a2_in, u * (D_HEAD * cs),
                            [(cs, D_HEAD), (CH2, n_cores), (1, cs)]),
                    in_=avt,
                )

            def cc1_and_vext(vext_all):
                # AllToAll #1 (v slices -> head owners), then gather to
                # vext_all[p, jt, u, 0:64]; jt = 2*src + i2 is a single
                # 32768-stride dim thanks to the chunk layout
                if no_collective:
                    for k in range(n_cores):
                        nc.sync.dma_start(out=a2a1_out[k], in_=a2a1_in[k])
                else:
                    nc.gpsimd.collective_compute(
                        "AllToAll",
                        mybir.AluOpType.bypass,
                        replica_groups=[list(range(n_cores))],
                        ins=[a2a1_in.opt()],
                        outs=[a2a1_out.opt()],
                    )
                for b in range(B):
                    for h in range(nh_loc):
                        nc.sync.dma_start(
                            out=vext_all[:, :, b * nh_loc + h, 0:D_HEAD],
                            in_=_ap(
                                a2a1_out,
                                b * (P * P) + h * D_HEAD,
                                [(P, P), (CH1 // tiles_pb, nt), (1, D_HEAD)],
                            ),
                        )

            def cc2_and_avg():
                # AllToAll #2 (normalized AV -> token owners), then gather to
                # avg_sb[p=(h,d), src, b*cs + tok]; (h,d) merges to one
                # 256-stride dim in the chunk layout
                if no_collective:
                    for k in range(n_cores):
                        nc.sync.dma_start(out=a2a2_out[k], in_=a2a2_in[k])
                else:
                    nc.gpsimd.collective_compute(
                        "AllToAll",
                        mybir.AluOpType.bypass,
                        replica_groups=[list(range(n_cores))],
                        ins=[a2a2_in.opt()],
                        outs=[a2a2_out.opt()],
                    )
                for b in range(B):
                    nc.sync.dma_start(
                        out=avg_sb[:, :, b * cs : (b + 1) * cs],
                        in_=_ap(a2a2_out, b * (nh_loc * D_HEAD * cs),
                                [(cs, P), (CH2, n_cores), (1, cs)]),
                    )

            if pipeline_out:
                # 3-stage cross-rep pipeline: the mega-loop of rep r runs
                # qkv(r) + attention(r-1) + out-proj(r-2); the collectives and
                # gathers for (r, r-1) run between mega-loops.  Rep 0 consumes
                # the zeroed bootstrap buffers; the trailing loops redo the
                # final rep's attention and the last two out-projs, so out_d
                # always ends correct (including repeat=1).
                for _rep in range(repeat):
                    qk_cur, qk_prev = qk_pk[_rep % 2], qk_pk[(_rep + 1) % 2]
                    vx_cur, vx_prev = vext_pk[_rep % 2], vext_pk[(_rep + 1) % 2]

                    def mega_body(i, qk_cur=qk_cur, qk_prev=qk_prev, vx_prev=vx_prev):
                        qkv_body(i, qk_cur)
                        attn_u_body(i, qk_prev, vx_prev)
                        out_body(i)

                    loop(tc, "qkv", n_g, "mega", mega_body)
                    cc1_and_vext(vx_cur)
                    cc2_and_avg()

                qk_last = qk_pk[(repeat - 1) % 2]
                vx_last = vext_pk[(repeat - 1) % 2]

                def tail_body(i, qk_last=qk_last, vx_last=vx_last):
                    attn_u_body(i, qk_last, vx_last)
                    out_body(i)

                loop(tc, "attn", n_u, "attn_tail", tail_body)
                cc2_and_avg()
                loop(tc, "out", n_it, "outproj", out_body)
            else:
                for _rep in range(repeat):
                    def qkv_only(g):
                        qkv_body(g, qk_pk[0])

                    def attn_only(u):
                        attn_u_body(u, qk_pk[0], vext_pk[0])

                    loop(tc, "qkv", n_g, "qkvproj", qkv_only)
                    cc1_and_vext(vext_pk[0])
                    loop(tc, "attn", n_u, "attn_u", attn_only)
                    cc2_and_avg()
                    loop(tc, "out", n_it, "outproj", out_body)

    nc.compile()
    return nc


def make_in_maps(h, Wq, Wkv, Wo, ln_g, ln_b, t=T_FULL, n_cores=N_CORES):
    """Builds the per-core input maps (host-side sharding/layout prep)."""
    bf = ml_dtypes.bfloat16
    nh_loc = N_HEAD // n_cores
    cs = t // n_cores
    n_it = B * cs // P
    n_g = t // IB

    # hq slabs (b, g): [p, k, tok] with d = k*128+p, token = g*512+tok
    hT = np.ascontiguousarray(h.transpose(1, 2, 0))  # [B, D, T]
    hq = hT.reshape(B, KCH, P, n_g, IB).transpose(3, 0, 2, 1, 4)  # [g, B, p, k, tok]
    hq = hq.transpose(0, 2, 1, 3, 4).reshape(n_g, P, B * KCH * IB)  # [g, p, (b,k,tok)]

    h_bmaj = np.ascontiguousarray(h.transpose(1, 0, 2)).reshape(B * t, D_MODEL)
    g = np.ascontiguousarray(ln_g, dtype=np.float32)
    bvec = np.ascontiguousarray(ln_b, dtype=np.float32)
    wo = np.ascontiguousarray(Wo).reshape(KCH, P, D_MODEL).astype(bf)
    wv_full = np.concatenate(
        [Wkv[:, hd * 2 * D_HEAD + D_HEAD : (hd + 1) * 2 * D_HEAD] for hd in range(N_HEAD)],
        axis=1,
    )
    wv = np.ascontiguousarray(wv_full.reshape(KCH, P, D_MODEL)).astype(bf)

    in_maps = []
    for c in range(n_cores):
        heads = [c * nh_loc + i for i in range(nh_loc)]
        cols = []
        for hd in heads:
            cols.append(Wq[:, hd * D_HEAD : (hd + 1) * D_HEAD] * SCALE)
            cols.append(Wkv[:, hd * 2 * D_HEAD : hd * 2 * D_HEAD + D_HEAD])
        wqk = np.concatenate(cols, axis=1)  # [1024, 256] = [q_h0|k_h0|q_h1|k_h1]
        hres = np.concatenate(
            [h_bmaj[b * t + c * cs : b * t + (c + 1) * cs] for b in range(B)]
        ).reshape(n_it, P, D_MODEL)
        # hv slabs vt' = 2*i2 + b: [p, k, tok], token = c*cs + i2*128 + tok
        hv = np.empty((B * 2, P, KCH * P), dtype=np.float32)
        for i2 in range(2):
            for b in range(B):
                sl = h[c * cs + i2 * P : c * cs + (i2 + 1) * P, b, :]  # [128 tok, D]
                slT = sl.T.reshape(KCH, P, P).transpose(1, 0, 2)  # [p, k, tok]
                hv[2 * i2 + b] = slT.reshape(P, KCH * P)
        # merged per-slab-group input: [g, p, qk slabs (b=0|1) + v slab]
        hqv = np.concatenate([hq, hv], axis=2).astype(bf)
        in_maps.append(
            {
                "hqv": hqv,
                "wqk": np.ascontiguousarray(wqk.reshape(KCH, P, 2 * nh_loc * D_HEAD)).astype(bf),
                "wv": wv,
                "wo": wo,
                "hres": np.ascontiguousarray(hres, dtype=np.float32),
                "lng": g,
                "lnb": bvec,
            }
        )
    return in_maps


def assemble_output(results, t=T_FULL, n_cores=N_CORES):
    cs = t // n_cores
    chunks = [results[c]["out"].reshape(B, cs, D_MODEL) for c in range(n_cores)]
    full = np.concatenate(chunks, axis=1)  # [B, t, D]
    return np.ascontiguousarray(full.transpose(1, 0, 2))


def _numpy_fallback(h, attn_mask, Wq, Wkv, Wo, ln_g, ln_b):
    t, b, _ = h.shape
    hf = h.reshape(t * b, D_MODEL)
    q = (hf @ Wq).reshape(t, b, N_HEAD, D_HEAD)
    kv = (hf @ Wkv).reshape(t, b, N_HEAD, 2 * D_HEAD)
    k, v = kv[..., :D_HEAD], kv[..., D_HEAD:]
    s = np.einsum("ibnd,jbnd->ijbn", q, k) * SCALE
    s = np.where(attn_mask[:, :, :, None], -np.inf, s)
    s = s - s.max(axis=1, keepdims=True)
    p = np.exp(s)
    p = p / p.sum(axis=1, keepdims=True)
    av = np.einsum("ijbn,jbnd->ibnd", p, v).reshape(t, b, N_HEAD * D_HEAD)
    ao = av @ Wo
    x = h + ao
    mu = x.mean(axis=-1, keepdims=True)
    var = ((x - mu) ** 2).mean(axis=-1, keepdims=True)
    return ((x - mu) / np.sqrt(var + LN_EPS) * ln_g + ln_b).astype(np.float32)


_PROGRAM_CACHE = {}


def kernel(h, attn_mask, Wq, Wkv, Wo, ln_g, ln_b):
    global LAST_RESULT
    h = np.asarray(h, dtype=np.float32)
    attn_mask = np.asarray(attn_mask)
    Wq = np.asarray(Wq, dtype=np.float32)
    Wkv = np.asarray(Wkv, dtype=np.float32)
    Wo = np.asarray(Wo, dtype=np.float32)
    ln_g = np.asarray(ln_g, dtype=np.float32)
    ln_b = np.asarray(ln_b, dtype=np.float32)

    t = h.shape[0]
    causal = np.triu(np.ones((t, t), dtype=bool), k=1)
    if not np.array_equal(attn_mask, np.broadcast_to(causal[:, :, None], attn_mask.shape)):
        return _numpy_fallback(h, attn_mask, Wq, Wkv, Wo, ln_g, ln_b)

    apply_gb = not (np.all(ln_g == 1.0) and np.all(ln_b == 0.0))
    key = (t, apply_gb)
    if key not in _PROGRAM_CACHE:
        _PROGRAM_CACHE[key] = build_program(t=t, apply_gb=apply_gb)
    nc = _PROGRAM_CACHE[key]

    in_maps = make_in_maps(h, Wq, Wkv, Wo, ln_g, ln_b, t=t)
    res = run_bass_kernel_spmd(
        nc,
        in_maps,
        core_ids=list(range(N_CORES)),
        trace=bool(int(os.environ.get("KERNEL_TRACE", "0"))),
    )
    LAST_RESULT = res
    return assemble_output(res.results, t=t)


if __name__ == "__main__":
    build_program()
    print("program built ok")


# revision 37
# speedup vs baseline: 1.0284x; 1.0284x over previous
"""Trainium2 Bass kernel for nn_MultiHeadAttn, v5: one pipelined hw loop.

The relay executing NEFFs in this environment charges wall time per
EMITTED instruction (decode/dispatch); executed loop iterations are
nearly free.  v4 wrapped every phase in its own tc.For_i (v3 left the
attention loop unrolled); v5 fuses all three phases into ONE mega-loop
via a 3-stage cross-rep software pipeline, paying the For_i framework
overhead (skip/reset/exit sync blocks, ~85 instructions) once instead
of three times:

  mega-loop (4 iters, rep r): qkv(r) + attention(r-1) + out-proj(r-2)
      qkv:  one DMA stages both batches' QK rhs slabs + the V lhsT
            slab; 48 matmuls; V results DMA'd straight into the
            AllToAll#1 input at a loop-affine offset.
      attn: full-partition dynamic-src copies stage qk/v, then a
            STATIC partition-shifted extraction of k (partition-
            shifted copies with a dynamic offset on EITHER side
            corrupt -- verified on hardware; symbolic matmul rhs
            costs +1 register-math ISA instruction per matmul);
            causal flash attention (40+40 matmuls, exp-bias trick,
            flash ones-column row sums); normalized AV DMA'd into the
            AllToAll#2 input at a loop-affine offset.  The attention
            matmul operands are staged in F32: an f32-ifmap Matmult is
            self-loading, so each of the 80 matmuls saves the separate
            Ldweights instruction the bf16 path would emit.
      out:  out-proj (16 matmuls) + residual + per-tile LN.
  between mega-loops: AllToAll#1(r) + vext gather, AllToAll#2(r-1) +
      avg gather.  qk_pack/vext are double-buffered (rep parity);
      trailing attn+out loops redo the final rep so out_d is always
      correct, including the real repeat=1 call.

Host-side layout prep makes every gather/scatter collapse to <=3-dim
DMA access patterns (the hard DMA limit) and every loop-varying offset
affine in the loop variable:
  hqv:  [4 groups g, 128, 2*KCH*512 + KCH*128]  (qk rhs + v lhsT slabs)
  a2a1 chunk layout [i2, b, tok, colh]: slab offset affine in vt'=2*i2+b,
      and jt = 2*src + i2 becomes a single 32768-stride gather dim.
  a2a2 chunk layout [u, d, tok]: offset affine in u; (h,d) partition
      dims merge into one 256-stride dim for the avg gather.
"""

import os
import sys

import numpy as np

try:
    import concourse.bass as bass  # noqa: F401
except ImportError:  # pragma: no cover
    sys.path.insert(0, "/opt/trn_rl_repo")

import ml_dtypes

import concourse.bass as bass
from concourse.bass import ds
import concourse.mybir as mybir
import concourse.tile as tile
from concourse import bacc
from concourse.bass_utils import run_bass_kernel_spmd
from concourse.masks import make_upper_triangular

# Problem constants
T_FULL = 2048
B = 2
D_MODEL = 1024
N_HEAD = 16
D_HEAD = 64
LN_EPS = 1e-5
N_CORES = 8
SCALE = 1.0 / (D_HEAD**0.5)
EXP_BIAS = -3.0  # scores are in [-3.3, 3.3] for this problem; keeps exp <= ~1.4

P = 128
KCH = D_MODEL // P  # 8 contraction chunks
IB = 512  # query block width
JG = 4  # j-tiles per score/exp group (group = [128, 4, 512] = 4 PSUM banks)

F32 = mybir.dt.float32
BF16 = mybir.dt.bfloat16

LAST_RESULT = None


def _ap(t, offset_elems, dims):
    """Hand-built access pattern: dims = [(stride, size), ...] in elements.

    offset_elems may be a loop-variable expression (symbolic AP)."""
    return bass.AP(tensor=t.tensor, offset=offset_elems + t.offset, ap=[list(d) for d in dims])


def build_program(t=T_FULL, n_cores=N_CORES, repeat=1, no_collective=False, apply_gb=True,
                  hw_loops=("qkv", "attn", "out"), pipeline_out=True,
                  ):
    nh_loc = N_HEAD // n_cores  # 2 heads per core
    assert nh_loc == 2 and B == 2
    nt = t // P  # 16 j-tiles per batch
    n_ib = t // IB  # 4 query blocks per batch
    n_g = t // IB  # 4 token slabs per batch for qk proj
    cs = t // n_cores  # 256 tokens per batch per core
    tiles_pb = cs // P  # 2 output tiles per batch
    n_it = B * tiles_pb  # 4 output tiles per core
    n_u = B * nh_loc  # 4 attention units per core
    SLAB_Q = P * KCH * IB  # elements per hq slab
    SLAB_V = P * KCH * P  # elements per hv slab
    CH1 = tiles_pb * B * P * P  # a2a1 per-chunk elements (65536)
    CH2 = n_u * D_HEAD * cs  # a2a2 per-chunk elements (65536)

    nc = bacc.Bacc("TRN2", target_bir_lowering=False, debug=False, num_devices=n_cores)

    def loop(tc, tag, n, name, body):
        if tag in hw_loops:
            with tc.For_i(0, n, name=name) as v:
                body(v)
        else:
            for v in range(n):
                body(v)

    # Kernel I/O
    # hqv: per token-block g, both batches' qk-proj rhs slabs followed by
    # the v-proj lhsT slab, so one in-loop DMA stages all matmul operands.
    QV_W = B * KCH * IB + KCH * P  # 9216 columns per slab group
    hqv_d = nc.dram_tensor("hqv", [n_g, P, QV_W], BF16, kind="ExternalInput").ap()
    wqk_d = nc.dram_tensor("wqk", [KCH, P, 2 * nh_loc * D_HEAD], BF16, kind="ExternalInput").ap()
    wv_d = nc.dram_tensor("wv", [KCH, P, D_MODEL], BF16, kind="ExternalInput").ap()
    wo_d = nc.dram_tensor("wo", [KCH, P, D_MODEL], BF16, kind="ExternalInput").ap()
    hres_d = nc.dram_tensor("hres", [n_it, P, D_MODEL], F32, kind="ExternalInput").ap()
    g_d = nc.dram_tensor("lng", [D_MODEL], F32, kind="ExternalInput").ap()
    b_d = nc.dram_tensor("lnb", [D_MODEL], F32, kind="ExternalInput").ap()
    out_d = nc.dram_tensor("out", [n_it, P, D_MODEL], F32, kind="ExternalOutput").ap()

    with tile.TileContext(nc) as tc:
        with (
            tc.tile_pool(name="consts", bufs=1) as consts,
            tc.tile_pool(name="sb", bufs=1) as sb,
            tc.tile_pool(name="pproj", bufs=1, space="PSUM") as pproj,
            tc.tile_pool(name="psc", bufs=1, space="PSUM") as psc,
            tc.tile_pool(name="pav", bufs=2, space="PSUM") as pav,
            tc.tile_pool(name="dram", bufs=1, space="DRAM") as dram,
        ):
            # ---- constants ----
            wqk_sb = consts.tile([P, KCH, 2 * nh_loc * D_HEAD], BF16)
            nc.sync.dma_start(
                out=wqk_sb,
                in_=_ap(wqk_d, 0, [(256, P), (P * 256, KCH), (1, 256)]),
            )
            wv_sb = consts.tile([P, KCH, D_MODEL], BF16)
            nc.sync.dma_start(
                out=wv_sb,
                in_=_ap(wv_d, 0, [(D_MODEL, P), (P * D_MODEL, KCH), (1, D_MODEL)]),
            )
            wo_sb = consts.tile([P, KCH, D_MODEL], BF16)
            nc.sync.dma_start(
                out=wo_sb,
                in_=_ap(wo_d, 0, [(D_MODEL, P), (P * D_MODEL, KCH), (1, D_MODEL)]),
            )
            if apply_gb:
                g_sb = consts.tile([P, D_MODEL], F32)
                b_sb = consts.tile([P, D_MODEL], F32)
                nc.sync.dma_start(out=g_sb, in_=_ap(g_d, 0, [(0, P), (1, D_MODEL)]))
                nc.sync.dma_start(out=b_sb, in_=_ap(b_d, 0, [(0, P), (1, D_MODEL)]))

            eps_sb = consts.tile([P, 1], F32)
            nc.vector.memset(eps_sb, LN_EPS)
            expb_sb = consts.tile([P, 1], F32)
            nc.vector.memset(expb_sb, EXP_BIAS)

            # grouped causal mask m4[:, g, :]: cols [0, g*128) = 0 (future keys),
            # [g*128, (g+1)*128) upper-triangular-incl-diag, rest = 1
            m4 = consts.tile([P, JG, IB], BF16)
            nc.gpsimd.memset(m4, 1.0)
            for g in range(1, JG):
                nc.gpsimd.memset(m4[:, g, 0 : g * P], 0.0)
            for g in range(JG):
                make_upper_triangular(nc, m4[:, g, g * P : (g + 1) * P], val=1.0, diag=True)

            # ---- tiles (declared once; buffers are reused across reps) ----
            a2a1_in = dram.tile([n_cores, tiles_pb, B, P, P], BF16, name="a2a1_in")
            a2a1_out = dram.tile([n_cores, tiles_pb, B, P, P], BF16, name="a2a1_out")
            a2a2_in = dram.tile([n_cores, n_u, D_HEAD, cs], BF16, name="a2a2_in")
            a2a2_out = dram.tile([n_cores, n_u, D_HEAD, cs], BF16, name="a2a2_out")

            # qk_pack: q on partitions 0:64, k on 64:128; flat (b, h) unit
            # dim.  Two explicit buffers: the mega-loop of rep r writes
            # qk_pk[r%2] while rep r-1's attention reads qk_pk[(r+1)%2].
            qk_pk = [sb.tile([P, n_u, t], BF16, tag=f"qkp{j}", name=f"qk_pack{j}")
                     for j in range(2)]
            vext_pk = [sb.tile([P, nt, n_u, D_HEAD + 1], BF16, tag=f"vext{j}",
                               name=f"vext_all{j}") for j in range(2)]
            hqv_t = sb.tile([P, QV_W], BF16, tag="hqv", name="hqv_t")
            vsm = sb.tile([P, D_MODEL], BF16, tag="vsm", name="vsm")
            V_OFF = B * KCH * IB  # v slab column offset within hqv_t

            # attention stages in F32: an f32-ifmap Matmult is self-loading,
            # so every f32 matmul saves the separate Ldweights instruction
            qkstage = sb.tile([P, t], F32, tag="qks", name="qkstage")
            kst = sb.tile([D_HEAD, t], F32, tag="kst", name="kst")
            vstage = sb.tile([P, nt, D_HEAD + 1], F32, tag="vst", name="vstage")
            avu = sb.tile([D_HEAD + 1, t], F32, tag="avu", name="avu")
            srow = sb.tile([1, t], F32, tag="srow", name="srow")
            rb = sb.tile([D_HEAD, t], F32, tag="rb", name="rb")
            avt = sb.tile([D_HEAD, t], BF16, tag="avt", name="avt")

            avg_sb = sb.tile([P, n_cores, B * cs], BF16, tag="avg", name="avg_sb")
            ostage = sb.tile([P, n_cores, P], BF16, tag="ost", name="ostage")
            hres_t = sb.tile([P, D_MODEL], F32, tag="hrt", name="hres_t")
            x_t = sb.tile([P, D_MODEL], F32, tag="x", name="x_t")
            # one tile for all LN scalars (fewer tags -> fewer per-loop
            # semaphore resets): [0:12] bn_stats x2, [12:14] (mean, var),
            # [14:15] std, [15:16] rstd
            lns = sb.tile([P, 16], F32, tag="lns", name="lns")

            # ones columns for the flash row-sum trick (gathers leave col 64
            # untouched), and defined values everywhere the pipelined rep-0
            # reads not-yet-produced data (results are recomputed by the
            # trailing loops, so out_d always ends correct)
            for j in range(2):
                nc.vector.memset(vext_pk[j], 0.0)
                nc.vector.memset(vext_pk[j][:, :, :, D_HEAD : D_HEAD + 1], 1.0)
                if pipeline_out:
                    nc.vector.memset(qk_pk[j], 0.0)
            if pipeline_out:
                nc.vector.memset(avg_sb, 0.0)

            def out_body(it):
                # tile it = (b, i2) covers avg cols [it*128, (it+1)*128)
                nc.vector.tensor_copy(ostage, avg_sb[:, :, ds(it * P, P)])
                nc.sync.dma_start(out=hres_t, in_=hres_d[ds(it, 1)])
                pos = pproj.tile([P, D_MODEL], F32, tag="proj", name="pos")
                for nh in range(2):
                    for k in range(n_cores):
                        nc.tensor.matmul(
                            pos[:, nh * IB : (nh + 1) * IB],
                            lhsT=ostage[:, k, :],
                            rhs=wo_sb[:, k, nh * IB : (nh + 1) * IB],
                            start=(k == 0),
                            stop=(k == n_cores - 1),
                        )
                nc.vector.tensor_add(x_t, pos, hres_t)
                for s in range(2):
                    nc.vector.bn_stats(lns[:, s * 6 : (s + 1) * 6], x_t[:, s * IB : (s + 1) * IB])
                nc.vector.bn_aggr(lns[:, 12:14], lns[:, 0:12])
                nc.scalar.activation(
                    lns[:, 14:15], lns[:, 13:14], mybir.ActivationFunctionType.Sqrt, bias=eps_sb
                )
                nc.vector.reciprocal(lns[:, 15:16], lns[:, 14:15])
                nc.vector.tensor_scalar(
                    out=x_t,
                    in0=x_t,
                    scalar1=lns[:, 12:13],
                    scalar2=lns[:, 15:16],
                    op0=mybir.AluOpType.subtract,
                    op1=mybir.AluOpType.mult,
                )
                if apply_gb:
                    nc.vector.tensor_mul(x_t, x_t, g_sb)
                    nc.vector.tensor_add(x_t, x_t, b_sb)
                nc.sync.dma_start(out=out_d[ds(it, 1)], in_=x_t)

            def qkv_body(g, qk_pack):
                # stage both batches' qk rhs slabs + the v lhsT slab, one DMA
                nc.sync.dma_start(
                    out=hqv_t,
                    in_=_ap(hqv_d, g * (P * QV_W), [(QV_W, P), (1, QV_W)]),
                )
                for b in range(B):
                    ps = pproj.tile([P, 2, IB], F32, tag="proj", name=f"ps_qk{b}")
                    # wqk columns are per-head [q|k] blocks, so ps[:, h, :]
                    # lands with q on partitions 0:64 and k on 64:128.
                    for mt in range(2):
                        for k in range(KCH):
                            nc.tensor.matmul(
                                ps[:, mt, :],
                                lhsT=wqk_sb[:, k, mt * P : (mt + 1) * P],
                                rhs=hqv_t[:, b * KCH * IB + k * IB : b * KCH * IB + (k + 1) * IB],
                                start=(k == 0),
                                stop=(k == KCH - 1),
                            )
                    nc.vector.tensor_copy(
                        qk_pack[:, b * nh_loc : (b + 1) * nh_loc, ds(g * IB, IB)],
                        ps,
                    )
                # V projection for slab vt' = g = 2*i2 + b
                psv = pproj.tile([P, 2, IB], F32, tag="proj", name="ps_v")
                for nh in range(2):
                    for k in range(KCH):
                        nc.tensor.matmul(
                            psv[:, nh, :],
                            lhsT=hqv_t[:, V_OFF + k * P : V_OFF + (k + 1) * P],
                            rhs=wv_sb[:, k, nh * IB : (nh + 1) * IB],
                            start=(k == 0),
                            stop=(k == KCH - 1),
                        )
                nc.vector.tensor_copy(vsm, psv)
                # straight into the AllToAll#1 input: chunk layout
                # [i2, b, tok, colh] -> slab offset = 16384 * vt'
                nc.sync.dma_start(
                    out=_ap(a2a1_in, g * (P * P),
                            [(P, P), (CH1, n_cores), (1, P)]),
                    in_=vsm,
                )

            def attn_u_body(u, qk_pack, vext_all):
                # full-partition dynamic-src stage, then STATIC partition-
                # shifted extraction of k.  Partition-shifted copies with a
                # dynamic offset on EITHER side corrupt (verified), and a
                # symbolic matmul rhs costs one register-math ISA
                # instruction per matmul, so q is read from the stage.
                nc.vector.tensor_copy(qkstage, qk_pack[:, ds(u, 1), :])
                nc.vector.tensor_copy(kst, qkstage[D_HEAD:P, :])
                nc.vector.tensor_copy(vstage, vext_all[:, :, ds(u, 1), :])
                for ib in range(n_ib):
                    avps = pav.tile([D_HEAD + 1, IB], F32, tag="av", name="avps")
                    for grp in range(ib + 1):
                        scp = psc.tile([P, JG, IB], F32, tag="sc", name="scp")
                        for jj in range(JG):
                            jt = JG * grp + jj
                            nc.tensor.matmul(
                                scp[:, jj, :],
                                lhsT=kst[:, jt * P : (jt + 1) * P],
                                rhs=qkstage[0:D_HEAD, ib * IB : (ib + 1) * IB],
                                start=True,
                                stop=True,
                            )
                        expt = sb.tile([P, JG, IB], F32, tag="exp", name="expt")
                        nc.scalar.activation(
                            expt, scp, mybir.ActivationFunctionType.Exp, bias=expb_sb
                        )
                        if grp == ib:
                            nc.vector.tensor_mul(expt, expt, m4)
                        for jj in range(JG):
                            jt = JG * grp + jj
                            nc.tensor.matmul(
                                avps,
                                lhsT=vstage[:, jt, :],
                                rhs=expt[:, jj, :],
                                start=(jt == 0),
                                stop=(jt == JG * ib + JG - 1),
                            )
                    nc.vector.tensor_copy(avu[:, ib * IB : (ib + 1) * IB], avps)
                # partition-shifted reads feed custom-DVE ops wrongly; a
                # plain tensor_copy handles the shift
                nc.vector.tensor_copy(srow, avu[D_HEAD : D_HEAD + 1, :])
                nc.vector.reciprocal_approx_fast(out=rb[0:1, :], in_=srow)
                nc.gpsimd.partition_broadcast(rb, rb[0:1, :])
                nc.vector.tensor_mul(avt, avu[0:D_HEAD, :], rb)
                # straight into the AllToAll#2 input: chunk layout
                # [u, d, tok] -> offset = u * D_HEAD * cs; token blocks of
                # 256 pair with destination cores
                nc.sync.dma_start(
                    out=_ap(a2a2_in, u * (D_HEAD * cs),
                            [(cs, D_HEAD), (CH2, n_cores), (1, cs)]),
                    in_=avt,
                )

            def cc1_and_vext(vext_all):
                # AllToAll #1 (v slices -> head owners), then gather to
                # vext_all[p, jt, u, 0:64]; jt = 2*src + i2 is a single
                # 32768-stride dim thanks to the chunk layout
                if no_collective:
                    for k in range(n_cores):
                        nc.sync.dma_start(out=a2a1_out[k], in_=a2a1_in[k])
                else:
                    nc.gpsimd.collective_compute(
                        "AllToAll",
                        mybir.AluOpType.bypass,
                        replica_groups=[list(range(n_cores))],
                        ins=[a2a1_in.opt()],
                        outs=[a2a1_out.opt()],
                    )
                for b in range(B):
                    for h in range(nh_loc):
                        nc.sync.dma_start(
                            out=vext_all[:, :, b * nh_loc + h, 0:D_HEAD],
                            in_=_ap(
                                a2a1_out,
                                b * (P * P) + h * D_HEAD,
                                [(P, P), (CH1 // tiles_pb, nt), (1, D_HEAD)],
                            ),
                        )

            def cc2_and_avg():
                # AllToAll #2 (normalized AV -> token owners), then gather to
                # avg_sb[p=(h,d), src, b*cs + tok]; (h,d) merges to one
                # 256-stride dim in the chunk layout
                if no_collective:
                    for k in range(n_cores):
                        nc.sync.dma_start(out=a2a2_out[k], in_=a2a2_in[k])
                else:
                    nc.gpsimd.collective_compute(
                        "AllToAll",
                        mybir.AluOpType.bypass,
                        replica_groups=[list(range(n_cores))],
                        ins=[a2a2_in.opt()],
                        outs=[a2a2_out.opt()],
                    )
                for b in range(B):
                    nc.sync.dma_start(
                        out=avg_sb[:, :, b * cs : (b + 1) * cs],
                        in_=_ap(a2a2_out, b * (nh_loc * D_HEAD * cs),
                                [(cs, P), (CH2, n_cores), (1, cs)]),
                    )

            if pipeline_out:
                # 3-stage cross-rep pipeline: the mega-loop of rep r runs
                # qkv(r) + attention(r-1) + out-proj(r-2); the collectives and
                # gathers for (r, r-1) run between mega-loops.  Rep 0 consumes
                # the zeroed bootstrap buffers; the trailing loops redo the
                # final rep's attention and the last two out-projs, so out_d
                # always ends correct (including repeat=1).
                for _rep in range(repeat):
                    qk_cur, qk_prev = qk_pk[_rep % 2], qk_pk[(_rep + 1) % 2]
                    vx_cur, vx_prev = vext_pk[_rep % 2], vext_pk[(_rep + 1) % 2]

                    def mega_body(i, qk_cur=qk_cur, qk_prev=qk_prev, vx_prev=vx_prev):
                        qkv_body(i, qk_cur)
                        attn_u_body(i, qk_prev, vx_prev)
                        out_body(i)

                    loop(tc, "qkv", n_g, "mega", mega_body)
                    cc1_and_vext(vx_cur)
                    cc2_and_avg()

                qk_last = qk_pk[(repeat - 1) % 2]
                vx_last = vext_pk[(repeat - 1) % 2]

                def tail_body(i, qk_last=qk_last, vx_last=vx_last):
                    attn_u_body(i, qk_last, vx_last)
                    out_body(i)

                loop(tc, "attn", n_u, "attn_tail", tail_body)
                cc2_and_avg()
                loop(tc, "out", n_it, "outproj", out_body)
            else:
                for _rep in range(repeat):
                    def qkv_only(g):
                        qkv_body(g, qk_pk[0])

                    def attn_only(u):
                        attn_u_body(u, qk_pk[0], vext_pk[0])

                    loop(tc, "qkv", n_g, "qkvproj", qkv_only)
                    cc1_and_vext(vext_pk[0])
                    loop(tc, "attn", n_u, "attn_u", attn_only)
                    cc2_and_avg()
                    loop(tc, "out", n_it, "outproj", out_body)

    nc.compile()
    return nc


def make_in_maps(h, Wq, Wkv, Wo, ln_g, ln_b, t=T_FULL, n_cores=N_CORES):
    """Builds the per-core input maps (host-side sharding/layout prep)."""
    bf = ml_dtypes.bfloat16
    nh_loc = N_HEAD // n_cores
    cs = t // n_cores
    n_it = B * cs // P
    n_g = t // IB

    # hq slabs (b, g): [p, k, tok] with d = k*128+p, token = g*512+tok
    hT = np.ascontiguousarray(h.transpose(1, 2, 0))  # [B, D, T]
    hq = hT.reshape(B, KCH, P, n_g, IB).transpose(3, 0, 2, 1, 4)  # [g, B, p, k, tok]
    hq = hq.transpose(0, 2, 1, 3, 4).reshape(n_g, P, B * KCH * IB)  # [g, p, (b,k,tok)]

    h_bmaj = np.ascontiguousarray(h.transpose(1, 0, 2)).reshape(B * t, D_MODEL)
    g = np.ascontiguousarray(ln_g, dtype=np.float32)
    bvec = np.ascontiguousarray(ln_b, dtype=np.float32)
    wo = np.ascontiguousarray(Wo).reshape(KCH, P, D_MODEL).astype(bf)
    wv_full = np.concatenate(
        [Wkv[:, hd * 2 * D_HEAD + D_HEAD : (hd + 1) * 2 * D_HEAD] for hd in range(N_HEAD)],
        axis=1,
    )
    wv = np.ascontiguousarray(wv_full.reshape(KCH, P, D_MODEL)).astype(bf)

    in_maps = []
    for c in range(n_cores):
        heads = [c * nh_loc + i for i in range(nh_loc)]
        cols = []
        for hd in heads:
            cols.append(Wq[:, hd * D_HEAD : (hd + 1) * D_HEAD] * SCALE)
            cols.append(Wkv[:, hd * 2 * D_HEAD : hd * 2 * D_HEAD + D_HEAD])
        wqk = np.concatenate(cols, axis=1)  # [1024, 256] = [q_h0|k_h0|q_h1|k_h1]
        hres = np.concatenate(
            [h_bmaj[b * t + c * cs : b * t + (c + 1) * cs] for b in range(B)]
        ).reshape(n_it, P, D_MODEL)
        # hv slabs vt' = 2*i2 + b: [p, k, tok], token = c*cs + i2*128 + tok
        hv = np.empty((B * 2, P, KCH * P), dtype=np.float32)
        for i2 in range(2):
            for b in range(B):
                sl = h[c * cs + i2 * P : c * cs + (i2 + 1) * P, b, :]  # [128 tok, D]
                slT = sl.T.reshape(KCH, P, P).transpose(1, 0, 2)  # [p, k, tok]
                hv[2 * i2 + b] = slT.reshape(P, KCH * P)
        # merged per-slab-group input: [g, p, qk slabs (b=0|1) + v slab]
        hqv = np.concatenate([hq, hv], axis=2).astype(bf)
        in_maps.append(
            {
                "hqv": hqv,
                "wqk": np.ascontiguousarray(wqk.reshape(KCH, P, 2 * nh_loc * D_HEAD)).astype(bf),
                "wv": wv,
                "wo": wo,
                "hres": np.ascontiguousarray(hres, dtype=np.float32),
                "lng": g,
                "lnb": bvec,
            }
        )
    return in_maps


def assemble_output(results, t=T_FULL, n_cores=N_CORES):
    cs = t // n_cores
    chunks = [results[c]["out"].reshape(B, cs, D_MODEL) for c in range(n_cores)]
    full = np.concatenate(chunks, axis=1)  # [B, t, D]
    return np.ascontiguousarray(full.transpose(1, 0, 2))


def _numpy_fallback(h, attn_mask, Wq, Wkv, Wo, ln_g, ln_b):
    t, b, _ = h.shape
    hf = h.reshape(t * b, D_MODEL)
    q = (hf @ Wq).reshape(t, b, N_HEAD, D_HEAD)
    kv = (hf @ Wkv).reshape(t, b, N_HEAD, 2 * D_HEAD)
    k, v = kv[..., :D_HEAD], kv[..., D_HEAD:]
    s = np.einsum("ibnd,jbnd->ijbn", q, k) * SCALE
    s = np.where(attn_mask[:, :, :, None], -np.inf, s)
    s = s - s.max(axis=1, keepdims=True)
    p = np.exp(s)
    p = p / p.sum(axis=1, keepdims=True)
    av = np.einsum("ijbn,jbnd->ibnd", p, v).reshape(t, b, N_HEAD * D_HEAD)
    ao = av @ Wo
    x = h + ao
    mu = x.mean(axis=-1, keepdims=True)
    var = ((x - mu) ** 2).mean(axis=-1, keepdims=True)
    return ((x - mu) / np.sqrt(var + LN_EPS) * ln_g + ln_b).astype(np.float32)


_PROGRAM_CACHE = {}


def kernel(h, attn_mask, Wq, Wkv, Wo, ln_g, ln_b):
    global LAST_RESULT
    h = np.asarray(h, dtype=np.float32)
    attn_mask = np.asarray(attn_mask)
    Wq = np.asarray(Wq, dtype=np.float32)
    Wkv = np.asarray(Wkv, dtype=np.float32)
    Wo = np.asarray(Wo, dtype=np.float32)
    ln_g = np.asarray(ln_g, dtype=np.float32)
    ln_b = np.asarray(ln_b, dtype=np.float32)

    t = h.shape[0]
    causal = np.triu(np.ones((t, t), dtype=bool), k=1)
    if not np.array_equal(attn_mask, np.broadcast_to(causal[:, :, None], attn_mask.shape)):
        return _numpy_fallback(h, attn_mask, Wq, Wkv, Wo, ln_g, ln_b)

    apply_gb = not (np.all(ln_g == 1.0) and np.all(ln_b == 0.0))
    key = (t, apply_gb)
    if key not in _PROGRAM_CACHE:
        _PROGRAM_CACHE[key] = build_program(t=t, apply_gb=apply_gb)
    nc = _PROGRAM_CACHE[key]

    in_maps = make_in_maps(h, Wq, Wkv, Wo, ln_g, ln_b, t=t)
    res = run_bass_kernel_spmd(
        nc,
        in_maps,
        core_ids=list(range(N_CORES)),
        trace=bool(int(os.environ.get("KERNEL_TRACE", "0"))),
    )
    LAST_RESULT = res
    return assemble_output(res.results, t=t)


if __name__ == "__main__":
    build_program()
    print("program built ok")
